# revision 1
# baseline (speedup 1.0000x reference)
"""Trainium2 Bass kernel for nn_ARCANetNew (GATv2 + GlobalAttention pooling).

Sharding: graphs (nodes / edges grouped by dst) split across 8 NeuronCores.
Each GAT layer computes xl = act(h)@Wl + bl locally per node, AllGathers the
xl table, and the edge phase gathers xl rows per edge (no per-edge recompute).
Self-loops are host-prepared ordinary edges (loop_attr in numpy). Pooling and
FC heads are graph-local.
"""
import os
import sys
import numpy as np

try:
    import concourse  # noqa: F401
except ImportError:
    sys.path.insert(0, '/opt/trn_rl_repo')

import ml_dtypes
import concourse.bass as bass
import concourse.bacc as bacc
import concourse.mybir as mybir
from concourse.tile import TileContext
from concourse.masks import make_identity

BF = mybir.dt.bfloat16
F32 = mybir.dt.float32
I32 = mybir.dt.int32
AF = mybir.ActivationFunctionType
OP = mybir.AluOpType

P = 128
NCORES = 8
GPC = 16            # graphs per core
EMB = 512
HEADS = 2
NL = 3
NPOOL = 4
N_NODES = 16384
N_EDGES = 65536
BATCH_G = 128
N_CONC = 32
GAT_SLOPE = 0.2
LR_SLOPE = 0.01
BN = float(np.sqrt(np.float32(1.0 + 1e-5)))
CLAMP = 60.0
KEXP = 17           # K-chunks for the 2089-wide gene matmul

_CACHE = {}


class _StopBuild(Exception):
    pass


def _bf(x):
    return np.asarray(x, np.float32).astype(ml_dtypes.bfloat16)


def _build(NBLK, ET, debug=False, stage=6):
    NSLOT = NBLK * P
    NT = NBLK * ET
    ELOC = NT * P
    NTAB = NCORES * NSLOT

    nc = bacc.Bacc("TRN2", target_bir_lowering=False, debug=False,
                   num_devices=NCORES)

    def din(name, shape, dt):
        return nc.dram_tensor(name, shape, dt, kind="ExternalInput")

    # per-core inputs
    xT0_d = din("xT0", [P, 2, NSLOT], BF)
    src_idx_d = din("src_idx", [P, NT], I32)
    dst_rel_d = din("dst_rel", [P, NT], F32)
    eaT_d = din("eaT", [10, ELOC], BF)
    G_d = din("G", [P, NBLK, 16], BF)
    GT_d = din("GT", [16, NSLOT], BF)
    exprT_d = din("exprT", [P, KEXP, 16], BF)
    concs_d = din("concs_a", [P, 4], F32)
    scal_d = din("scal", [P, 16], F32)
    # replicated weights
    w0l_d = din("w0l", [P, 2, 512], BF)
    w0r_d = din("w0r", [P, 2, 512], BF)
    b0l_d = din("b0l", [1, 512], BF)
    b0r_d = din("b0r", [1, 512], BF)
    bias0_d = din("bias0", [1, 512], BF)
    we0_d = din("we0", [10, 512], BF)
    att0_d = din("att0", [P, 512], BF)
    wl_d = din("wl", [P, NL * 4, 1024], BF)
    wr_d = din("wr", [P, NL * 4, 1024], BF)
    bl_d = din("bl", [1, NL * 1024], BF)
    br_d = din("br", [1, NL * 1024], BF)
    biasg_d = din("biasg", [1, NL * 512], BF)
    weg_d = din("weg", [10, NL, 1024], BF)
    attg_d = din("attg", [P, NL, 1024], BF)
    geneW_d = din("geneW", [P, KEXP, 512], BF)
    geneb_d = din("geneb", [1, 512], BF)
    pgW1_d = din("pgW1", [P, 16, 1024], BF)
    pnW1_d = din("pnW1", [P, 16, 1024], BF)
    pnW2_d = din("pnW2", [P, 32, 512], BF)
    pgW2_d = din("pgW2", [P, 32], BF)
    pgb1T_d = din("pgb1T", [P, 32], F32)
    pnb1T_d = din("pnb1T", [P, 32], F32)
    pnb2_d = din("pnb2", [4, 512], BF)
    fsW1_d = din("fsW1", [P, 4, 1024], BF)
    fsW2_d = din("fsW2", [P, 8, 512], BF)
    fsb1_d = din("fsb1", [1, 1024], BF)
    fsb2_d = din("fsb2", [1, 512], BF)
    fbW1_d = din("fbW1", [P, 4, 1024], BF)
    fbW2_d = din("fbW2", [P, 8, 512], BF)
    fbb1_d = din("fbb1", [1, 1024], BF)
    fbb2_d = din("fbb2", [1, 512], BF)
    fW1_d = din("fW1", [P, 4, 2048], BF)
    fb1_d = din("fb1", [1, 2048], BF)
    fW2_d = din("fW2", [P, 16, 512], BF)
    fb2_d = din("fb2", [1, 512], BF)
    WoR_d = din("WoR", [P, 512], F32)
    Rconc_d = din("Rconc", [16, 512], BF)

    y_d = nc.dram_tensor("y", [P, 4], F32, kind="ExternalOutput")
    if debug:
        hdump_d = nc.dram_tensor("hdump", [NSLOT, 512], F32,
                                 kind="ExternalOutput")
        gfdump_d = nc.dram_tensor("gfdump", [16, 512], F32,
                                  kind="ExternalOutput")

    ccw = [512, 1024, 1024, 1024]
    cc_in = [nc.dram_tensor(f"cc_in{l}", [NSLOT, ccw[l]], BF)
             for l in range(4)]
    t_dram = nc.dram_tensor("t_dram", [2, NSLOT, 512], BF)
    cc_out = [nc.dram_tensor(f"cc_out{l}", [NTAB, ccw[l]], BF,
                             addr_space="Shared") for l in range(4)]
    nocc = bool(os.environ.get("K_NOCC"))

    with TileContext(nc) as tc:
      try:
        with tc.tile_pool(name="glob", bufs=1) as gp, \
             tc.tile_pool(name="f32s", bufs=5) as fp, \
             tc.tile_pool(name="fsml", bufs=2) as fs, \
             tc.tile_pool(name="pwork", bufs=3, space="PSUM") as pw, \
             tc.tile_pool(name="ptp", bufs=1, space="PSUM") as pt, \
             tc.tile_pool(name="paccum", bufs=1, space="PSUM") as pa:

            # ---------- constants / global residents ----------
            id_bf = gp.tile([P, P], BF, tag="id_bf")
            make_identity(nc, id_bf[:])
            id_f32 = gp.tile([P, P], F32, tag="id_f32")
            make_identity(nc, id_f32[:])
            ones_row = gp.tile([1, P], BF, tag="ones_row")
            nc.vector.memset(ones_row[:], 1.0)
            ones_row512 = gp.tile([1, 512], BF, tag="ones_row512")
            nc.vector.memset(ones_row512[:], 1.0)
            ones_col = gp.tile([P, 1], BF, tag="ones_col")
            nc.vector.memset(ones_col[:], 1.0)
            iota_i = gp.tile([P, P], I32, tag="iota_i")
            nc.gpsimd.iota(out=iota_i[:], pattern=[[1, P]], base=0,
                           channel_multiplier=0)
            iota_row = gp.tile([P, P], F32, tag="iota_row")
            nc.vector.tensor_copy(out=iota_row[:], in_=iota_i[:])
            iota_c = gp.tile([P, 1], F32, tag="iota_c")

            psio = pt.tile([P, P], F32, tag="tp")
            nc.tensor.transpose(out=psio[:], in_=iota_row[:],
                                identity=id_f32[:])
            nc.vector.tensor_copy(out=iota_c[:], in_=psio[:, 0:1])
            scal = gp.tile([P, 16], F32, tag="scal")
            nc.sync.dma_start(out=scal[:], in_=scal_d[:])
            sig = gp.tile([P, 16], F32, tag="sig")
            nc.scalar.activation(out=sig[:], in_=scal[:], func=AF.Sigmoid)
            one_m = gp.tile([P, 16], F32, tag="one_m")
            nc.scalar.activation(out=one_m[:], in_=scal[:], func=AF.Sigmoid,
                                 scale=-1.0)

            G = gp.tile([P, NBLK, 16], BF, tag="G")
            nc.sync.dma_start(out=G[:], in_=G_d[:])
            GT = gp.tile([16, NSLOT], BF, tag="GT")
            nc.sync.dma_start(out=GT[:], in_=GT_d[:])
            h_loc = gp.tile([P, NBLK, 512], F32, tag="h_loc")
            rg_f = gp.tile([16, 512], F32, tag="rg_f")
            rg_bf = gp.tile([16, 512], BF, tag="rg_bf")
            gf_acc = gp.tile([16, 512], F32, tag="gf_acc")

            # ---------- gene encoder (scoped) ----------
            with tc.tile_pool(name="genep", bufs=1) as gep:
                exprT = gep.tile([P, KEXP, 16], BF, tag="exprT")
                nc.sync.dma_start(out=exprT[:], in_=exprT_d[:])
                geneW = gep.tile([P, KEXP, 512], BF, tag="geneW")
                nc.sync.dma_start(out=geneW[:], in_=geneW_d[:])
                geneb = gep.tile([1, 512], BF, tag="geneb")
                nc.sync.dma_start(out=geneb[:], in_=geneb_d[:])
                psg = pw.tile([16, 512], F32, tag="work")
                for k in range(KEXP):
                    nc.tensor.matmul(out=psg[:], lhsT=exprT[:, k, :],
                                     rhs=geneW[:, k, :], start=(k == 0),
                                     stop=False)
                nc.tensor.matmul(out=psg[:], lhsT=ones_row[:1, :16],
                                 rhs=geneb[:1, :], start=False, stop=True)
                nc.scalar.activation(out=rg_f[:], in_=psg[:], func=AF.Relu)
                nc.vector.tensor_copy(out=rg_bf[:], in_=rg_f[:])

            # ---------- GAT phase ----------
            with tc.tile_pool(name="gatres", bufs=1) as rp, \
                 tc.tile_pool(name="gatw", bufs=1) as wp, \
                 tc.tile_pool(name="gscr", bufs=4) as sp, \
                 tc.tile_pool(name="gnode", bufs=3) as np_:

                srcI = rp.tile([P, NT], I32, tag="srcI")
                nc.sync.dma_start(out=srcI[:], in_=src_idx_d[:])
                dstR = rp.tile([P, NT], F32, tag="dstR")
                nc.sync.dma_start(out=dstR[:], in_=dst_rel_d[:])
                xiT = rp.tile([P, 4, NSLOT], BF, tag="xiT")
                xr_loc = rp.tile([P, NBLK, 1024], BF, tag="xr_loc")
                we0 = rp.tile([10, 512], BF, tag="we0")
                nc.sync.dma_start(out=we0[:], in_=we0_d[:])
                att0 = rp.tile([P, 512], BF, tag="att0")
                nc.sync.dma_start(out=att0[:], in_=att0_d[:])
                weg = rp.tile([10, NL, 1024], BF, tag="weg")
                nc.sync.dma_start(out=weg[:], in_=weg_d[:])
                attg = rp.tile([P, NL, 1024], BF, tag="attg")
                nc.sync.dma_start(out=attg[:], in_=attg_d[:])
                b0l = rp.tile([1, 512], BF, tag="b0l")
                nc.sync.dma_start(out=b0l[:], in_=b0l_d[:])
                b0r = rp.tile([1, 512], BF, tag="b0r")
                nc.sync.dma_start(out=b0r[:], in_=b0r_d[:])
                bias0 = rp.tile([1, 512], BF, tag="bias0")
                nc.sync.dma_start(out=bias0[:], in_=bias0_d[:])
                blr = rp.tile([1, NL * 1024], BF, tag="blr")
                nc.sync.dma_start(out=blr[:], in_=bl_d[:])
                brr = rp.tile([1, NL * 1024], BF, tag="brr")
                nc.sync.dma_start(out=brr[:], in_=br_d[:])
                biasg = rp.tile([1, NL * 512], BF, tag="biasg")
                nc.sync.dma_start(out=biasg[:], in_=biasg_d[:])
                bias_rep = rp.tile([P, 512], F32, tag="bias_rep")
                hge = rp.tile([P, 1], F32, tag="hge")

                for l in range(min(4, stage)):
                    H = 1 if l == 0 else 2
                    Kc = 2 if l == 0 else 4
                    W = H * 512

                    # -- node phase: xl -> cc_in[l], xr -> xr_loc --
                    if l == 0:
                        myxiT = wp.tile([P, 2, NSLOT], BF, tag="xT0t")
                        nc.sync.dma_start(out=myxiT[:], in_=xT0_d[:])
                        Wl = wp.tile([P, 2, 512], BF, tag="w0l")
                        nc.sync.dma_start(out=Wl[:], in_=w0l_d[:])
                        Wr = wp.tile([P, 2, 512], BF, tag="w0r")
                        nc.sync.dma_start(out=Wr[:], in_=w0r_d[:])
                        Wl_s = lambda k, h: Wl[:, k, :]
                        Wr_s = lambda k, h: Wr[:, k, :]
                        bl_s = lambda h: b0l[:1, :]
                        br_s = lambda h: b0r[:1, :]
                        We_s = lambda h: we0[:, :]
                        att_s = lambda h: att0[:, :]
                    else:
                        li = l - 1
                        Wl = wp.tile([P, 4, 1024], BF, tag="wlL")
                        nc.sync.dma_start(out=Wl[:],
                                          in_=wl_d[:, li * 4:(li + 1) * 4, :])
                        Wr = wp.tile([P, 4, 1024], BF, tag="wrL")
                        nc.sync.dma_start(out=Wr[:],
                                          in_=wr_d[:, li * 4:(li + 1) * 4, :])
                        Wl_s = lambda k, h: Wl[:, k, h * 512:(h + 1) * 512]
                        Wr_s = lambda k, h: Wr[:, k, h * 512:(h + 1) * 512]
                        bl_s = lambda h, li=li: blr[:1, li * 1024
                                    + h * 512:li * 1024 + (h + 1) * 512]
                        br_s = lambda h, li=li: brr[:1, li * 1024
                                    + h * 512:li * 1024 + (h + 1) * 512]
                        We_s = lambda h, li=li: weg[:, li,
                                                    h * 512:(h + 1) * 512]
                        att_s = lambda h, li=li: attg[:, li,
                                                      h * 512:(h + 1) * 512]
                        # xiT = lrelu(h).T
                        myxiT = xiT
                        for k in range(4):
                            for b in range(NBLK):
                                psf = pw.tile([P, P], F32, tag="work")
                                nc.tensor.transpose(
                                    out=psf[:],
                                    in_=h_loc[:, b, k * P:(k + 1) * P],
                                    identity=id_f32[:])
                                nc.scalar.activation(
                                    out=xiT[:, k, b * P:(b + 1) * P],
                                    in_=psf[:], func=AF.Prelu, alpha=LR_SLOPE)

                    for b in range(NBLK):
                        for h in range(H):
                            psn = pw.tile([P, 512], F32, tag="work")
                            for k in range(Kc):
                                nc.tensor.matmul(
                                    out=psn[:],
                                    lhsT=myxiT[:, k, b * P:(b + 1) * P],
                                    rhs=Wl_s(k, h), start=(k == 0),
                                    stop=False)
                            nc.tensor.matmul(out=psn[:], lhsT=ones_row[:1, :],
                                             rhs=bl_s(h), start=False,
                                             stop=True)
                            xl_t = np_.tile([P, 512], BF, tag="xl_t")
                            nc.vector.tensor_copy(out=xl_t[:], in_=psn[:])
                            nc.sync.dma_start(
                                out=cc_in[l][b * P:(b + 1) * P,
                                             h * 512:(h + 1) * 512],
                                in_=xl_t[:])

                    if not nocc:
                        nc.gpsimd.collective_compute(
                            "AllGather", OP.bypass, ins=[cc_in[l][:]],
                            outs=[cc_out[l][:]],
                            replica_groups=[list(range(NCORES))])
                    else:
                        nc.sync.dma_start(out=cc_out[l][0:NSLOT, :],
                                          in_=cc_in[l][:])
                    tab = cc_out[l]

                    for b in range(NBLK):
                        for h in range(H):
                            psr = pw.tile([P, 512], F32, tag="work")
                            for k in range(Kc):
                                nc.tensor.matmul(
                                    out=psr[:],
                                    lhsT=myxiT[:, k, b * P:(b + 1) * P],
                                    rhs=Wr_s(k, h), start=(k == 0),
                                    stop=False)
                            nc.tensor.matmul(out=psr[:], lhsT=ones_row[:1, :],
                                             rhs=br_s(h), start=False,
                                             stop=True)
                            nc.scalar.copy(
                                out=xr_loc[:, b, h * 512:(h + 1) * 512],
                                in_=psr[:])

                    # per-layer epilogue scalars
                    psb = pw.tile([P, 512], F32, tag="work")
                    if l == 0:
                        nc.tensor.matmul(out=psb[:], lhsT=ones_row[:1, :],
                                         rhs=bias0[:1, :], start=True,
                                         stop=True)
                        nc.scalar.copy(out=bias_rep[:], in_=psb[:])
                    else:
                        nc.tensor.matmul(out=psb[:], lhsT=ones_row[:1, :],
                                         rhs=biasg[:1, (l - 1) * 512:l * 512],
                                         start=True, stop=True)
                        nc.vector.tensor_scalar(out=bias_rep[:], in0=psb[:],
                                                scalar1=one_m[:, l - 1:l],
                                                scalar2=None, op0=OP.mult)
                        nc.vector.tensor_scalar(out=hge[:, :1],
                                                in0=one_m[:, l - 1:l],
                                                scalar1=0.5, scalar2=None,
                                                op0=OP.mult)

                    # -- edge phase --
                    for b in range(NBLK):
                        agg = [pa.tile([P, 512], F32, tag=f"agg{h}",
                                       name=f"agg{h}") for h in range(H)]
                        ps_s = [pa.tile([P, 1], F32, tag=f"ps_s{h}",
                                        name=f"ps_s{h}") for h in range(H)]
                        for t in range(ET):
                            gt = b * ET + t
                            xi = sp.tile([P, W], BF, tag="xi_g")
                            nc.gpsimd.indirect_dma_start(
                                out=xi[:], out_offset=None, in_=tab[:],
                                in_offset=bass.IndirectOffsetOnAxis(
                                    ap=srcI[:, gt:gt + 1], axis=0))
                            ea_st = sp.tile([10, P], BF, tag="ea_st")
                            nc.sync.dma_start(
                                out=ea_st[:],
                                in_=eaT_d[:, gt * P:(gt + 1) * P])
                            se = sp.tile([P, P], BF, tag="sel_en")
                            nc.vector.tensor_scalar(
                                out=se[:], in0=iota_row[:],
                                scalar1=dstR[:, gt:gt + 1], scalar2=None,
                                op0=OP.is_equal)
                            psx = pt.tile([P, P], BF, tag="tp")
                            nc.tensor.transpose(out=psx[:], in_=se[:],
                                                identity=id_bf[:])
                            sn = sp.tile([P, P], BF, tag="sel_ne")
                            nc.vector.tensor_copy(out=sn[:], in_=psx[:])

                            lgt = fs.tile([P, 2], F32, tag="lgt")
                            for h in range(H):
                                pm = pw.tile([P, 512], F32, tag="work")
                                nc.tensor.matmul(
                                    out=pm[:], lhsT=sn[:],
                                    rhs=xr_loc[:, b, h * 512:(h + 1) * 512],
                                    start=True, stop=False)
                                nc.tensor.matmul(
                                    out=pm[:], lhsT=ea_st[:],
                                    rhs=We_s(h), start=False, stop=False,
                                    skip_group_check=True)
                                nc.tensor.matmul(
                                    out=pm[:], lhsT=id_bf[:],
                                    rhs=xi[:, h * 512:(h + 1) * 512],
                                    start=False, stop=True,
                                    skip_group_check=True)
                                lr = sp.tile([P, 512], BF, tag="lr")
                                nc.scalar.activation(out=lr[:], in_=pm[:],
                                                     func=AF.Prelu,
                                                     alpha=GAT_SLOPE)
                                scr = sp.tile([P, 512], BF, tag="scr")
                                nc.vector.tensor_tensor(out=scr[:], in0=lr[:],
                                                        in1=att_s(h),
                                                        op=OP.mult)
                                nc.vector.tensor_reduce(
                                    out=lgt[:, h:h + 1], in_=scr[:],
                                    axis=mybir.AxisListType.X, op=OP.add)
                            lgc = fs.tile([P, 2], F32, tag="lgc")
                            nc.vector.tensor_scalar(out=lgc[:, :H],
                                                    in0=lgt[:, :H],
                                                    scalar1=CLAMP,
                                                    scalar2=None, op0=OP.min)
                            e_t = fs.tile([P, 2], F32, tag="e_t")
                            nc.scalar.activation(out=e_t[:, :H],
                                                 in_=lgc[:, :H], func=AF.Exp)
                            for h in range(H):
                                sa = sp.tile([P, P], BF, tag="sel_a")
                                nc.vector.tensor_scalar(
                                    out=sa[:], in0=se[:],
                                    scalar1=e_t[:, h:h + 1], scalar2=None,
                                    op0=OP.mult)
                                nc.tensor.matmul(
                                    out=agg[h][:], lhsT=sa[:],
                                    rhs=xi[:, h * 512:(h + 1) * 512],
                                    start=(t == 0), stop=(t == ET - 1))
                                nc.tensor.matmul(out=ps_s[h][:], lhsT=sa[:],
                                                 rhs=ones_col[:],
                                                 start=(t == 0),
                                                 stop=(t == ET - 1))
                        # block epilogue
                        rsm = fs.tile([P, 2], F32, tag="rsm")
                        for h in range(H):
                            nc.vector.tensor_scalar(out=rsm[:, h:h + 1],
                                                    in0=ps_s[h][:],
                                                    scalar1=1e-20,
                                                    scalar2=None, op0=OP.max)
                        rs = fs.tile([P, 2], F32, tag="rs")
                        nc.vector.reciprocal(out=rs[:, :H], in_=rsm[:, :H])
                        if l > 0:
                            nc.vector.tensor_scalar(out=rs[:, :H],
                                                    in0=rs[:, :H],
                                                    scalar1=hge[:, :1],
                                                    scalar2=None, op0=OP.mult)
                        if l == 0:
                            u0 = fp.tile([P, 512], F32, tag="fa")
                            nc.vector.tensor_scalar(out=u0[:], in0=agg[0][:],
                                                    scalar1=rs[:, 0:1],
                                                    scalar2=None, op0=OP.mult)
                            nc.vector.tensor_tensor(out=h_loc[:, b, :],
                                                    in0=u0[:],
                                                    in1=bias_rep[:],
                                                    op=OP.add)
                        else:
                            u0 = fp.tile([P, 512], F32, tag="fa")
                            nc.vector.tensor_scalar(out=u0[:], in0=agg[0][:],
                                                    scalar1=rs[:, 0:1],
                                                    scalar2=None, op0=OP.mult)
                            u1 = fp.tile([P, 512], F32, tag="fa")
                            nc.vector.tensor_scalar(out=u1[:], in0=agg[1][:],
                                                    scalar1=rs[:, 1:2],
                                                    scalar2=None, op0=OP.mult)
                            hsum = fp.tile([P, 512], F32, tag="fa")
                            nc.vector.tensor_tensor(out=hsum[:], in0=u1[:],
                                                    in1=u0[:], op=OP.add)
                            t2a = fp.tile([P, 512], F32, tag="fa")
                            nc.vector.tensor_scalar(out=t2a[:],
                                                    in0=h_loc[:, b, :],
                                                    scalar1=sig[:, l - 1:l],
                                                    scalar2=None, op0=OP.mult)
                            t2 = fp.tile([P, 512], F32, tag="fa")
                            nc.vector.tensor_tensor(out=t2[:], in0=t2a[:],
                                                    in1=bias_rep[:],
                                                    op=OP.add)
                            nc.vector.tensor_tensor(out=h_loc[:, b, :],
                                                    in0=hsum[:], in1=t2[:],
                                                    op=OP.add)

            if debug:
                for b in range(NBLK):
                    nc.sync.dma_start(out=hdump_d[b * P:(b + 1) * P, :],
                                      in_=h_loc[:, b, :])

            # ---------- pooling ----------
            if stage < 5:
                raise _StopBuild()
            nc.vector.tensor_scalar(out=gf_acc[:], in0=rg_f[:],
                                    scalar1=float(NPOOL), scalar2=None,
                                    op0=OP.mult)
            grps = []
            s0 = 0
            while s0 < NBLK:
                w = min(4, NBLK - s0)
                grps.append((s0, w))
                s0 += w

            with tc.tile_pool(name="poolres", bufs=1) as pr, \
                 tc.tile_pool(name="poolw", bufs=1) as ppw, \
                 tc.tile_pool(name="pscr", bufs=2) as ps2, \
                 tc.tile_pool(name="plrn", bufs=1) as psl, \
                 tc.tile_pool(name="pscr3", bufs=2) as ps3:
                # shared across pools: vT_raw0 (h^T), vT_lr{0,1}, v_nm{0,1}
                vT_raw0 = pr.tile([P, 4, NSLOT], BF, tag="vT_raw0")
                vT_lr0 = pr.tile([P, 4, NSLOT], BF, tag="vT_lr0")
                vT_lr1 = pr.tile([P, 4, NSLOT], BF, tag="vT_lr1")
                v_nm0 = pr.tile([P, NBLK, 512], BF, tag="v_nm0")
                v_nm1 = pr.tile([P, NBLK, 512], BF, tag="v_nm1")
                rgT = pr.tile([P, 4, 16], BF, tag="rgT")
                for b in range(NBLK):
                    nc.scalar.copy(out=v_nm0[:, b, :], in_=h_loc[:, b, :])
                    prg = pw.tile([P, 512], F32, tag="work")
                    nc.tensor.matmul(out=prg[:],
                                     lhsT=GT[:, b * P:(b + 1) * P],
                                     rhs=rg_bf[:], start=True, stop=True)
                    nc.vector.tensor_tensor(out=v_nm1[:, b, :],
                                            in0=h_loc[:, b, :], in1=prg[:],
                                            op=OP.add)
                for k in range(4):
                    for b in range(NBLK):
                        psf = pw.tile([P, P], F32, tag="work")
                        nc.tensor.transpose(
                            out=psf[:], in_=h_loc[:, b, k * P:(k + 1) * P],
                            identity=id_f32[:])
                        nc.scalar.copy(out=vT_raw0[:, k, b * P:(b + 1) * P],
                                       in_=psf[:])
                        nc.scalar.activation(
                            out=vT_lr0[:, k, b * P:(b + 1) * P], in_=psf[:],
                            func=AF.Prelu, alpha=LR_SLOPE)
                        psx = pw.tile([P, P], BF, tag="work")
                        nc.tensor.transpose(
                            out=psx[:], in_=v_nm1[:, b, k * P:(k + 1) * P],
                            identity=id_bf[:])
                        nc.scalar.activation(
                            out=vT_lr1[:, k, b * P:(b + 1) * P], in_=psx[:],
                            func=AF.Prelu, alpha=LR_SLOPE)
                    psb3 = pt.tile([P, 16], BF, tag="tp")
                    nc.tensor.transpose(out=psb3[:],
                                        in_=rg_bf[:, k * P:(k + 1) * P],
                                        identity=id_bf[:16, :16])
                    nc.scalar.copy(out=rgT[:, k, :], in_=psb3[:])

                for pi in range(NPOOL):
                    gW1 = ppw.tile([P, 4, 1024], BF, tag="gW1")
                    nc.sync.dma_start(out=gW1[:],
                                      in_=pgW1_d[:, pi * 4:(pi + 1) * 4, :])
                    nW1 = ppw.tile([P, 4, 1024], BF, tag="nW1")
                    nc.sync.dma_start(out=nW1[:],
                                      in_=pnW1_d[:, pi * 4:(pi + 1) * 4, :])
                    nW2 = ppw.tile([P, 8, 512], BF, tag="nW2")
                    nc.sync.dma_start(out=nW2[:],
                                      in_=pnW2_d[:, pi * 8:(pi + 1) * 8, :])
                    gW2 = ppw.tile([P, 8], BF, tag="gW2")
                    nc.sync.dma_start(out=gW2[:],
                                      in_=pgW2_d[:, pi * 8:(pi + 1) * 8])
                    gb1T = ppw.tile([P, 8], F32, tag="gb1T")
                    nc.sync.dma_start(out=gb1T[:],
                                      in_=pgb1T_d[:, pi * 8:(pi + 1) * 8])
                    nb1T = ppw.tile([P, 8], F32, tag="nb1T")
                    nc.sync.dma_start(out=nb1T[:],
                                      in_=pnb1T_d[:, pi * 8:(pi + 1) * 8])
                    nb2 = ppw.tile([1, 512], BF, tag="nb2")
                    nc.sync.dma_start(out=nb2[:], in_=pnb2_d[pi:pi + 1, :])
                    # rgW = rg @ nW1  [16, 1024]
                    rgW = ppw.tile([16, 1024], BF, tag="rgW")
                    for half in range(2):
                        psrg = pw.tile([16, 512], F32, tag="work")
                        for k in range(4):
                            nc.tensor.matmul(
                                out=psrg[:], lhsT=rgT[:, k, :],
                                rhs=nW1[:, k, half * 512:(half + 1) * 512],
                                start=(k == 0), stop=(k == 3))
                        nc.scalar.copy(out=rgW[:, half * 512:(half + 1) * 512],
                                       in_=psrg[:])

                    # phase A: per group — nn W1 (shared Z + rank-16
                    # correction), nn W2 into t_all, gate W1 logits for both
                    # inputs into psglog0/1.
                    psglog = [pa.tile([P, NBLK], F32, tag=f"agg{vi}",
                                      name=f"agg{vi}") for vi in range(2)]
                    for (g0, gw) in grps:
                        Wn = gw * P
                        lrn = [psl.tile([P, 8, 512], BF, tag=f"lrn{vi}",
                                        name=f"lrn{vi}")
                               for vi in range(2)]
                        for blk in range(8):
                            pn1 = pw.tile([P, 512], F32, tag="work")
                            for k in range(4):
                                nc.tensor.matmul(
                                    out=pn1[:, :Wn],
                                    lhsT=nW1[:, k, blk * P:(blk + 1) * P],
                                    rhs=vT_raw0[:, k, g0 * P:g0 * P + Wn],
                                    start=(k == 0), stop=(k == 3))
                            nc.scalar.activation(
                                out=lrn[0][:, blk, :Wn],
                                in_=pn1[:, :Wn], func=AF.Prelu,
                                alpha=LR_SLOPE, bias=nb1T[:, blk:blk + 1])
                            nc.tensor.matmul(
                                out=pn1[:, :Wn],
                                lhsT=rgW[:, blk * P:(blk + 1) * P],
                                rhs=GT[:, g0 * P:g0 * P + Wn], start=False,
                                stop=True, skip_group_check=True)
                            nc.scalar.activation(
                                out=lrn[1][:, blk, :Wn],
                                in_=pn1[:, :Wn], func=AF.Prelu,
                                alpha=LR_SLOPE, bias=nb1T[:, blk:blk + 1])
                        for vi in range(2):
                            vT_lr = vT_lr1 if vi else vT_lr0
                            lrmg = ps2.tile([P, 8, 512], BF, tag="big8")
                            for blk in range(8):
                                pmg = pw.tile([P, 512], F32, tag="work")
                                for k in range(4):
                                    nc.tensor.matmul(
                                        out=pmg[:, :Wn],
                                        lhsT=gW1[:, k, blk * P:(blk + 1) * P],
                                        rhs=vT_lr[:, k, g0 * P:g0 * P + Wn],
                                        start=(k == 0), stop=(k == 3))
                                nc.scalar.activation(
                                    out=lrmg[:, blk, :Wn], in_=pmg[:, :Wn],
                                    func=AF.Prelu, alpha=LR_SLOPE,
                                    bias=gb1T[:, blk:blk + 1])
                            for j in range(gw):
                                nt = g0 + j
                                for blk in range(8):
                                    nc.tensor.matmul(
                                        out=psglog[vi][:, nt:nt + 1],
                                        lhsT=lrmg[:, blk, j * P:(j + 1) * P],
                                        rhs=gW2[:, blk:blk + 1],
                                        start=(blk == 0), stop=(blk == 7))
                            for j in range(gw):
                                nt = g0 + j
                                pt2 = pw.tile([P, 512], F32, tag="work")
                                for blk in range(8):
                                    nc.tensor.matmul(
                                        out=pt2[:],
                                        lhsT=lrn[vi][:, blk,
                                                     j * P:(j + 1) * P],
                                        rhs=nW2[:, blk, :],
                                        start=(blk == 0), stop=False)
                                nc.tensor.matmul(out=pt2[:],
                                                 lhsT=ones_row[:1, :],
                                                 rhs=nb2[:1, :], start=False,
                                                 stop=True)
                                t_sb = ps3.tile([P, 512], BF, tag="t_sb")
                                nc.vector.tensor_copy(out=t_sb[:],
                                                      in_=pt2[:])
                                nc.sync.dma_start(
                                    out=t_dram[vi, nt * P:(nt + 1) * P, :],
                                    in_=t_sb[:])

                    # phase B: softmax + weighted sums per input
                    for vi in range(2):
                        v_nm = v_nm1 if vi else v_nm0
                        gl1 = fs.tile([P, NBLK], F32, tag="gl1")
                        nc.vector.tensor_scalar(
                            out=gl1[:], in0=psglog[vi][:],
                            scalar1=scal[:, 10 + pi:11 + pi], scalar2=CLAMP,
                            op0=OP.add, op1=OP.min)
                        e_col = fs.tile([P, NBLK], F32, tag="e_col")
                        nc.scalar.activation(out=e_col[:], in_=gl1[:],
                                             func=AF.Exp)
                        e_bf = ps3.tile([P, NBLK], BF, tag="e_bf")
                        nc.vector.tensor_copy(out=e_bf[:], in_=e_col[:])
                        pssg = pa.tile([16, 1], F32, tag="ps_s0")
                        for b in range(NBLK):
                            nc.tensor.matmul(out=pssg[:], lhsT=G[:, b, :],
                                             rhs=e_bf[:, b:b + 1],
                                             start=(b == 0),
                                             stop=(b == NBLK - 1))
                        rs_g = fs.tile([16, 1], F32, tag="rs_g")
                        nc.vector.reciprocal(out=rs_g[:], in_=pssg[:])

                        psU_v = pa.tile([16, 512], F32, tag="ps_s1",
                                        name="psU_v")
                        psU_t = pa.tile([16, 512], F32, tag="agg0",
                                        name="psU_t")
                        for nt in range(NBLK):
                            Gw = ps3.tile([P, 16], BF, tag="Gw")
                            nc.vector.tensor_scalar(
                                out=Gw[:], in0=G[:, nt, :],
                                scalar1=e_col[:, nt:nt + 1], scalar2=None,
                                op0=OP.mult)
                            nc.tensor.matmul(out=psU_v[:], lhsT=Gw[:],
                                             rhs=v_nm[:, nt, :],
                                             start=(nt == 0),
                                             stop=(nt == NBLK - 1))
                            t_st = ps3.tile([P, 512], BF, tag="t_st")
                            nc.sync.dma_start(
                                out=t_st[:],
                                in_=t_dram[vi, nt * P:(nt + 1) * P, :])
                            nc.tensor.matmul(out=psU_t[:], lhsT=Gw[:],
                                             rhs=t_st[:],
                                             start=(nt == 0),
                                             stop=(nt == NBLK - 1))
                        un1 = fp.tile([16, 512], F32, tag="fa")
                        nc.vector.tensor_scalar(out=un1[:], in0=psU_t[:],
                                                scalar1=one_m[:16,
                                                              3 + pi:4 + pi],
                                                scalar2=None, op0=OP.mult)
                        un2a = fp.tile([16, 512], F32, tag="fa")
                        nc.vector.tensor_scalar(out=un2a[:], in0=psU_v[:],
                                                scalar1=sig[:16,
                                                            3 + pi:4 + pi],
                                                scalar2=None, op0=OP.mult)
                        un2 = fp.tile([16, 512], F32, tag="fa")
                        nc.vector.tensor_tensor(out=un2[:], in0=un2a[:],
                                                in1=un1[:], op=OP.add)
                        un3 = fp.tile([16, 512], F32, tag="fa")
                        nc.vector.tensor_scalar(out=un3[:], in0=un2[:],
                                                scalar1=rs_g[:16, :1],
                                                scalar2=None, op0=OP.mult)
                        nc.vector.tensor_tensor(out=gf_acc[:], in0=gf_acc[:],
                                                in1=un3[:], op=OP.add)

            if debug:
                nc.sync.dma_start(out=gfdump_d[:], in_=gf_acc[:])

            # ---------- FC heads + concentration head ----------
            if stage < 6:
                raise _StopBuild()
            with tc.tile_pool(name="fcres", bufs=1) as fr, \
                 tc.tile_pool(name="fscr", bufs=3) as fsc:
                gf_bf = fr.tile([16, 512], BF, tag="gf_bf")
                nc.vector.tensor_copy(out=gf_bf[:], in_=gf_acc[:])
                gfT = fr.tile([P, 4, 16], BF, tag="gfT")
                for k in range(4):
                    psb3 = pt.tile([P, 16], BF, tag="tp")
                    nc.tensor.transpose(out=psb3[:],
                                        in_=gf_bf[:, k * P:(k + 1) * P],
                                        identity=id_bf[:16, :16])
                    nc.scalar.copy(out=gfT[:, k, :], in_=psb3[:])

                sb_vec = []
                for (W1d, W2d, b1d, b2d, gcol) in (
                        (fsW1_d, fsW2_d, fsb1_d, fsb2_d, 7),
                        (fbW1_d, fbW2_d, fbb1_d, fbb2_d, 8)):
                    W1 = fsc.tile([P, 4, 1024], BF, tag="hW1")
                    nc.sync.dma_start(out=W1[:], in_=W1d[:])
                    W2 = fsc.tile([P, 8, 512], BF, tag="hW2")
                    nc.sync.dma_start(out=W2[:], in_=W2d[:])
                    b1 = fsc.tile([1, 1024], BF, tag="hb1")
                    nc.sync.dma_start(out=b1[:], in_=b1d[:])
                    b2 = fsc.tile([1, 512], BF, tag="hb2")
                    nc.sync.dma_start(out=b2[:], in_=b2d[:])
                    l1 = fsc.tile([16, 1024], BF, tag="l1")
                    for half in range(2):
                        ps1 = pw.tile([16, 512], F32, tag="work")
                        for k in range(4):
                            nc.tensor.matmul(
                                out=ps1[:], lhsT=gfT[:, k, :],
                                rhs=W1[:, k, half * 512:(half + 1) * 512],
                                start=(k == 0), stop=False)
                        nc.tensor.matmul(
                            out=ps1[:], lhsT=ones_row[:1, :16],
                            rhs=b1[:1, half * 512:(half + 1) * 512],
                            start=False, stop=True)
                        nc.scalar.activation(
                            out=l1[:, half * 512:(half + 1) * 512],
                            in_=ps1[:], func=AF.Prelu, alpha=LR_SLOPE)
                    l1T = fsc.tile([P, 8, 16], BF, tag="l1T")
                    for j in range(8):
                        psb4 = pw.tile([P, 16], BF, tag="work")
                        nc.tensor.transpose(out=psb4[:],
                                            in_=l1[:, j * P:(j + 1) * P],
                                            identity=id_bf[:16, :16])
                        nc.scalar.copy(out=l1T[:, j, :], in_=psb4[:])
                    ps2_ = pw.tile([16, 512], F32, tag="work")
                    for j in range(8):
                        nc.tensor.matmul(out=ps2_[:], lhsT=l1T[:, j, :],
                                         rhs=W2[:, j, :], start=(j == 0),
                                         stop=False)
                    nc.tensor.matmul(out=ps2_[:], lhsT=ones_row[:1, :16],
                                     rhs=b2[:1, :], start=False, stop=True)
                    v1 = fp.tile([16, 512], F32, tag="fa")
                    nc.vector.tensor_scalar(out=v1[:], in0=gf_acc[:],
                                            scalar1=sig[:16, gcol:gcol + 1],
                                            scalar2=None, op0=OP.mult)
                    sv = fr.tile([16, 512], BF, tag=f"sv{gcol}")
                    v2a = fp.tile([16, 512], F32, tag="fa")
                    nc.vector.tensor_scalar(out=v2a[:], in0=ps2_[:],
                                            scalar1=one_m[:16, gcol:gcol + 1],
                                            scalar2=None, op0=OP.mult)
                    v2 = fp.tile([16, 512], F32, tag="fa")
                    nc.vector.tensor_tensor(out=v2[:], in0=v2a[:], in1=v1[:],
                                            op=OP.add)
                    nc.vector.tensor_scalar(out=sv[:], in0=v2[:],
                                            scalar1=float(1.0 / BN),
                                            scalar2=None, op0=OP.mult)
                    sb_vec.append(sv)
                s_bf, b_bf = sb_vec

                Rcon = fr.tile([16, 512], BF, tag="Rcon")
                nc.sync.dma_start(out=Rcon[:], in_=Rconc_d[:])
                ccs = fr.tile([P, 4], F32, tag="ccs")
                nc.sync.dma_start(out=ccs[:], in_=concs_d[:])
                cc_ar = fr.tile([P, 4], F32, tag="cc_ar")
                nc.vector.tensor_scalar(out=cc_ar[:], in0=ccs[:], scalar1=0.0,
                                        scalar2=-1.0, op0=OP.max, op1=OP.mult)
                fW1 = fr.tile([P, 4, 2048], BF, tag="fW1c")
                nc.sync.dma_start(out=fW1[:], in_=fW1_d[:])
                fW2 = fr.tile([P, 16, 512], BF, tag="fW2c")
                nc.sync.dma_start(out=fW2[:], in_=fW2_d[:])
                fb1 = fr.tile([1, 2048], BF, tag="fb1c")
                nc.sync.dma_start(out=fb1[:], in_=fb1_d[:])
                fb2 = fr.tile([1, 512], BF, tag="fb2c")
                nc.sync.dma_start(out=fb2[:], in_=fb2_d[:])
                WoR = fr.tile([P, 512], F32, tag="WoR")
                nc.sync.dma_start(out=WoR[:], in_=WoR_d[:])
                yo = fr.tile([P, 4], F32, tag="yo")

                for k in range(4):
                    psB = pw.tile([P, 512], F32, tag="work")
                    nc.tensor.matmul(out=psB[:],
                                     lhsT=Rcon[:, k * P:(k + 1) * P],
                                     rhs=b_bf[:], start=True, stop=True)
                    psS = pw.tile([P, 512], F32, tag="work")
                    nc.tensor.matmul(out=psS[:],
                                     lhsT=Rcon[:, k * P:(k + 1) * P],
                                     rhs=s_bf[:], start=True, stop=True)
                    zta = fp.tile([P, 512], F32, tag="fa")
                    nc.vector.tensor_scalar(out=zta[:], in0=psS[:],
                                            scalar1=cc_ar[:, k:k + 1],
                                            scalar2=None, op0=OP.mult)
                    zt = fp.tile([P, 512], F32, tag="fa")
                    nc.vector.tensor_tensor(out=zt[:], in0=zta[:], in1=psB[:],
                                            op=OP.add)
                    u_f = fsc.tile([P, 512], F32, tag="u_f")
                    nc.scalar.activation(out=u_f[:], in_=zt[:],
                                         func=AF.Sigmoid)
                    u_bf = fsc.tile([P, 512], BF, tag="u_bf")
                    nc.vector.tensor_copy(out=u_bf[:], in_=u_f[:])
                    uT = fsc.tile([P, 4, P], BF, tag="uT")
                    for kk in range(4):
                        psx3 = pw.tile([P, P], BF, tag="work")
                        nc.tensor.transpose(out=psx3[:],
                                            in_=u_bf[:, kk * P:(kk + 1) * P],
                                            identity=id_bf[:])
                        nc.scalar.copy(out=uT[:, kk, :], in_=psx3[:])
                    r1 = fsc.tile([P, 2048], BF, tag="r1")
                    for q in range(4):
                        pr1 = pw.tile([P, 512], F32, tag="work")
                        for kk in range(4):
                            nc.tensor.matmul(
                                out=pr1[:], lhsT=uT[:, kk, :],
                                rhs=fW1[:, kk, q * 512:(q + 1) * 512],
                                start=(kk == 0), stop=False)
                        nc.tensor.matmul(out=pr1[:], lhsT=ones_row[:1, :],
                                         rhs=fb1[:1, q * 512:(q + 1) * 512],
                                         start=False, stop=True)
                        nc.scalar.activation(out=r1[:, q * 512:(q + 1) * 512],
                                             in_=pr1[:], func=AF.Relu)
                    r1T = fsc.tile([P, 16, P], BF, tag="r1T")
                    for j in range(16):
                        psx4 = pw.tile([P, P], BF, tag="work")
                        nc.tensor.transpose(out=psx4[:],
                                            in_=r1[:, j * P:(j + 1) * P],
                                            identity=id_bf[:])
                        nc.scalar.copy(out=r1T[:, j, :], in_=psx4[:])
                    pst = pw.tile([P, 512], F32, tag="work")
                    for j in range(16):
                        nc.tensor.matmul(out=pst[:], lhsT=r1T[:, j, :],
                                         rhs=fW2[:, j, :], start=(j == 0),
                                         stop=False)
                    nc.tensor.matmul(out=pst[:], lhsT=ones_row[:1, :],
                                     rhs=fb2[:1, :], start=False, stop=True)
                    t_f = fp.tile([P, 512], F32, tag="fa")
                    nc.scalar.activation(out=t_f[:], in_=pst[:],
                                         func=AF.Sigmoid)
                    w1 = fp.tile([P, 512], F32, tag="fa")
                    nc.vector.tensor_scalar(out=w1[:], in0=u_f[:],
                                            scalar1=sig[:, 9:10],
                                            scalar2=None, op0=OP.mult)
                    w2 = fp.tile([P, 512], F32, tag="fa")
                    nc.vector.tensor_scalar(out=w2[:], in0=t_f[:],
                                            scalar1=one_m[:, 9:10],
                                            scalar2=None, op0=OP.mult)
                    v_f = fp.tile([P, 512], F32, tag="fa")
                    nc.vector.tensor_tensor(out=v_f[:], in0=w1[:], in1=w2[:],
                                            op=OP.add)
                    scr3 = fp.tile([P, 512], F32, tag="fa")
                    nc.vector.tensor_tensor(out=scr3[:], in0=v_f[:],
                                            in1=WoR[:], op=OP.mult)
                    ycol = fs.tile([P, 1], F32, tag="ycol")
                    nc.vector.tensor_reduce(
                        out=ycol[:, :1], in_=scr3[:],
                        axis=mybir.AxisListType.X, op=OP.add)
                    nc.scalar.activation(out=yo[:, k:k + 1], in_=ycol[:],
                                         func=AF.Identity,
                                         bias=scal[:, 14:15])
                nc.sync.dma_start(out=y_d[:], in_=yo[:])
      except _StopBuild:
        pass

    nc.compile()
    return nc


def _prep(inputs, NBLK, ET):
    """Host-side sharding. Returns in_maps or None if padding too small."""
    NSLOT = NBLK * P
    NT = NBLK * ET
    ELOC = NT * P
    i = {}
    for k, v in inputs.items():
        a = np.asarray(v)
        if a.dtype == np.float64:
            a = a.astype(np.float32)
        i[k] = a
    batch = i['batch'].astype(np.int64)
    ei = i['edge_index'].astype(np.int64)
    src_g, dst_g = ei[0], ei[1]
    ea = i['edge_attr'].astype(np.float32)

    # self-loop attrs: mean of incoming edge_attr (PyG fill_value='mean')
    deg_g = np.bincount(dst_g, minlength=N_NODES).astype(np.float32)
    loop_attr = np.zeros((N_NODES, 10), np.float32)
    np.add.at(loop_attr, dst_g, ea)
    loop_attr /= np.maximum(deg_g, 1.0)[:, None]

    bounds = np.searchsorted(batch, np.arange(0, BATCH_G + 1, GPC))
    n0s, n1s = bounds[:-1], bounds[1:]
    cnts = n1s - n0s
    if cnts.max() > NSLOT:
        return None
    node_core = np.searchsorted(bounds[1:], np.arange(N_NODES), side='right')

    # Balance edge counts (incl. one self-edge per node) across node-blocks
    # per core (greedy FFD on in-degree+1).
    ecap = ET * P
    slot_maps = []
    for c in range(NCORES):
        n0, n1 = n0s[c], n1s[c]
        cnt = n1 - n0
        emask_c = (dst_g >= n0) & (dst_g < n1)
        degl = np.bincount(dst_g[emask_c] - n0, minlength=cnt) + 1
        order_n = np.argsort(-degl, kind='stable')
        blk_e = np.zeros(NBLK, np.int64)
        blk_n = np.zeros(NBLK, np.int64)
        slot_of = np.full(cnt, -1, np.int64)
        ok = True
        for nl in order_n:
            d = degl[nl]
            cand = np.where((blk_n < P) & (blk_e + d <= ecap))[0]
            if len(cand) == 0:
                ok = False
                break
            bsel = cand[np.argmin(blk_e[cand])]
            slot_of[nl] = bsel * P + blk_n[bsel]
            blk_n[bsel] += 1
            blk_e[bsel] += d
        if not ok:
            return None
        slot_maps.append(slot_of)
    row_of_node = np.zeros(N_NODES, np.int64)
    for c in range(NCORES):
        n0, n1 = n0s[c], n1s[c]
        row_of_node[n0:n1] = c * NSLOT + slot_maps[c]
    _prep.last_slot_maps = slot_maps
    _prep.last_bounds = bounds

    def chunkmaj(w, kc):
        K, N = w.shape
        pad = kc * P - K
        if pad:
            w = np.concatenate([w, np.zeros((pad, N), w.dtype)], 0)
        return np.ascontiguousarray(_bf(w.reshape(kc, P, N).transpose(1, 0, 2)))

    rep = {}
    rep['w0l'] = chunkmaj(i['g0_Wl'].astype(np.float32), 2)
    rep['w0r'] = chunkmaj(i['g0_Wr'].astype(np.float32), 2)
    rep['b0l'] = _bf(i['g0_bl'][None, :])
    rep['b0r'] = _bf(i['g0_br'][None, :])
    rep['bias0'] = _bf(i['g0_bias'][None, :])
    rep['we0'] = _bf(i['g0_We'])
    rep['att0'] = np.ascontiguousarray(
        np.broadcast_to(_bf(i['g0_att'].reshape(1, 512)), (P, 512)))
    wl_c = np.stack([chunkmaj(i['g_Wl'][l], 4) for l in range(NL)], 1)
    rep['wl'] = np.ascontiguousarray(wl_c.reshape(P, NL * 4, 1024))
    wr_c = np.stack([chunkmaj(i['g_Wr'][l], 4) for l in range(NL)], 1)
    rep['wr'] = np.ascontiguousarray(wr_c.reshape(P, NL * 4, 1024))
    rep['bl'] = _bf(i['g_bl'].reshape(1, -1))
    rep['br'] = _bf(i['g_br'].reshape(1, -1))
    rep['biasg'] = _bf(i['g_bias'].reshape(1, -1))
    rep['weg'] = np.ascontiguousarray(_bf(i['g_We']).transpose(1, 0, 2))
    rep['attg'] = np.ascontiguousarray(np.broadcast_to(
        _bf(i['g_att'].reshape(NL, 1, HEADS * EMB)),
        (NL, P, HEADS * EMB)).transpose(1, 0, 2))
    gw = i['gene_W'].astype(np.float32)
    gw = np.concatenate(
        [gw, np.zeros((KEXP * P - gw.shape[0], EMB), np.float32)], 0)
    rep['geneW'] = np.ascontiguousarray(
        _bf(gw.reshape(KEXP, P, EMB).transpose(1, 0, 2)))
